# revision 7
# baseline (speedup 1.0000x reference)
"""Trainium2 Bass kernel: 15x15 valid cross-correlation of 4096x4096 (+bias).

Sharding: output columns split across 8 NeuronCores (512 cols/core; each core
gets a [4096, 526] column slab of x — 14-col halo), gathered on the host.

Per core the conv runs as banded-Toeplitz matmuls on the TensorEngine:
for each of 36 row-blocks (M=114 out rows, K=M+14=128 input rows on the
partition dim) accumulate 15 matmuls into one PSUM bank —

    psum[:M, :512] += T_kw.T @ x_blk[:, kw : kw+512]      kw = 0..14

where T_kw[h_in, h_out] = wt[h_in - h_out, kw] holds the kh taps as a band
and the kw tap is a free-dim offset into the same SBUF tile (no data
duplication). The 15 Toeplitz matrices are built host-side, cast to bf16
(matmuls accumulate fp32 in PSUM; measured rel err ~2e-3); the bias rides
the PSUM->SBUF drain on the VectorEngine. Measured ~137 us on hardware:
TensorE streams 540 back-to-back matmuls at ~216 ns (N=512 @ 2.4 GHz) with
zero gaps; the remainder is NEFF preamble + the Tile exit barrier.
"""

import numpy as np

H = 4096
W = 4096
KH = 15
KW = 15
OH = H - KH + 1  # 4082
OW = W - KW + 1  # 4082
NCORES = 8
COLS = 512              # output cols per core
INC = COLS + KW - 1     # 526
BLK = 114               # output rows per row-block
NBLK = (OH + BLK - 1) // BLK  # 36 (last block M=92)

_CACHE = {}


def _build_program():
    import concourse.tile as tile
    from concourse import bacc, mybir
    from contextlib import ExitStack

    nc = bacc.Bacc("TRN2", target_bir_lowering=False, debug=False,
                   num_devices=NCORES)
    bf16 = mybir.dt.bfloat16
    f32 = mybir.dt.float32
    x_d = nc.dram_tensor("x", [H, INC], bf16, kind="ExternalInput").ap()
    w_d = nc.dram_tensor("wt", [128, KW * BLK], bf16,
                         kind="ExternalInput").ap()
    b_d = nc.dram_tensor("bias", [128, 1], f32, kind="ExternalInput").ap()
    o_d = nc.dram_tensor("out", [OH, COLS], f32, kind="ExternalOutput").ap()

    with ExitStack() as ctx:
        tc = ctx.enter_context(tile.TileContext(nc))
        wpool = ctx.enter_context(tc.tile_pool(name="wp", bufs=1))
        bpool = ctx.enter_context(tc.tile_pool(name="bp", bufs=1))
        xpool = ctx.enter_context(tc.tile_pool(name="xp", bufs=3))
        opool = ctx.enter_context(tc.tile_pool(name="op", bufs=3))
        pspool = ctx.enter_context(tc.tile_pool(name="ps", bufs=2, space="PSUM"))

        # weight + bias dispatch on the gpsimd sequencer so they overlap the
        # x-block dispatches on the sync sequencer
        wt_t = wpool.tile([128, KW * BLK], bf16)
        nc.gpsimd.dma_start(wt_t[:], w_d[:])
        b_t = bpool.tile([128, 1], f32)
        nc.gpsimd.dma_start(b_t[:], b_d[:])

        # warm the HAM clock gate during the DMA-bound startup window: dummy
        # full-width matmuls on a zeroed scratch tile retire before block 0's
        # data lands, so the real matmul stream starts at 2.4 GHz
        scr = wpool.tile([128, COLS], bf16, tag="scr")
        nc.vector.memset(scr[:], 0.0)
        wps = pspool.tile([8, COLS], f32, tag="warm")
        for _ in range(10):
            nc.tensor.matmul(wps[:, :], scr[:, :8], scr[:, :],
                             start=True, stop=True, skip_group_check=True)

        for b in range(NBLK):
            r0 = b * BLK
            m = min(BLK, OH - r0)
            k = m + KH - 1
            x_t = xpool.tile([128, INC], bf16)
            nc.sync.dma_start(x_t[:k, :], x_d[r0:r0 + k, :])
            # last block: split the free dim so drain+store pipeline with MMs
            nh = 2 if b == NBLK - 1 else 1
            nw = COLS // nh
            for h in range(nh):
                ps = pspool.tile([BLK, nw], f32, tag=f"ps{nh}{h}")
                for kw in range(KW):
                    nc.tensor.matmul(
                        ps[:m, :],
                        wt_t[:k, kw * BLK: kw * BLK + m],
                        x_t[:k, h * nw + kw: h * nw + kw + nw],
                        start=(kw == 0),
                        stop=(kw == KW - 1),
                    )
                o_t = opool.tile([BLK, nw], f32, tag=f"o{nh}{h}")
                nc.vector.tensor_scalar_add(o_t[:m, :], ps[:m, :], b_t[:m, :])
                nc.sync.dma_start(o_d[r0:r0 + m, h * nw: (h + 1) * nw],
                                  o_t[:m, :])

    nc.compile()
    return nc


def _toeplitz(weight):
    wtoep = np.zeros((128, KW * BLK), np.float32)
    idx = np.arange(BLK)
    for kw in range(KW):
        for d in range(KH):  # d = h_in - h_out
            wtoep[idx + d, kw * BLK + idx] = weight[d, kw]
    return wtoep


def _prepare_in_maps(x, weight, bias):
    import ml_dtypes
    x = np.asarray(x, dtype=np.float32)
    weight = np.asarray(weight, dtype=np.float32)
    bias = np.asarray(bias, dtype=np.float32)

    x_pad = np.zeros((H, NCORES * COLS + KW - 1), np.float32)
    x_pad[:, :W] = x
    x_bf = x_pad.astype(ml_dtypes.bfloat16)
    wtoep = _toeplitz(weight).astype(ml_dtypes.bfloat16)
    bias_b = np.full((128, 1), bias.reshape(-1)[0], np.float32)

    in_maps = []
    for c in range(NCORES):
        shard = np.ascontiguousarray(x_bf[:, c * COLS: c * COLS + INC])
        in_maps.append({"x": shard, "wt": wtoep, "bias": bias_b})
    return in_maps


def _run(x, weight, bias, trace=False):
    from concourse.bass_utils import run_bass_kernel_spmd

    if "nc" not in _CACHE:
        _CACHE["nc"] = _build_program()
    nc = _CACHE["nc"]

    in_maps = _prepare_in_maps(x, weight, bias)
    res = run_bass_kernel_spmd(nc, in_maps, core_ids=list(range(NCORES)),
                               trace=trace)
    out = np.empty((OH, NCORES * COLS), np.float32)
    for c in range(NCORES):
        out[:, c * COLS: (c + 1) * COLS] = res.results[c]["out"]
    return out[:, :OW], res


def kernel(x, weight, bias):
    out, _ = _run(x, weight, bias, trace=False)
    return out
